# revision 1
# baseline (speedup 1.0000x reference)
"""Trainium2 Bass kernel for nn_MeshLoss2D (chamfer distance between a point
cloud and a bilinearly-refined mesh).

Contract: kernel(vertices, pc) takes FULL inputs, returns the FULL (scalar)
output. Internally shards across 8 NeuronCores:

  chamfer = mean_p min_q d(p,q) + mean_q min_p d(p,q),
  d(p,q) = |a_p|^2 + |b_q|^2 - 2 a_p . b_q

The full distance d(p,q) (including both norm terms) is computed on-device by
a K=13 bf16 matmul with hi/lo-split augmented vectors (compensated bf16: the
a.b product keeps hi*hi + hi*lo + lo*hi cross terms, and each norm term is
carried as a bf16 hi+lo pair, so d matches fp32 to ~5e-4 absolute). Distances
stream through PSUM in [128, 2048] groups; each group is consumed by one of
three routes, statically balanced across the engines:

  D: DVE reduce_min straight from PSUM (fp32-exact, 1 elem/cycle/lane)
  A: ACT copies PSUM -> fp16 SBUF, then DVE pairwise tensor_tensor-min ops
     (2 results/cycle) tree-combine the slices per row-tile

Since d >= 0 and near-min values are small, the fp16 egress rounds at ~2.4e-4
relative. Per-row-tile partial mins land in a strip that a final reduce_min
collapses; the host only computes means (O(P+Q) work). Every core runs the
same program on 1/8 of the queries of both min-directions and both batches.
"""

import sys

sys.path.insert(0, "/opt/trn_rl_repo")

import ml_dtypes
import numpy as np

import concourse.mybir as mybir
from concourse import bacc
from concourse.bass_utils import run_bass_kernel_spmd
from concourse.tile import TileContext

# ---- problem constants (hardcoded; kernel.py must be self-contained) ----
N_BATCH = 2
P = 8192                # point-cloud points per batch
Q = 95 * 95             # 9025 refined mesh points per batch
N_CORES = 8
KDIM = 13               # augmentation slots (hi/lo split product + both norms)

Q_PAD = 9216            # mesh points padded (= 72*128 = 18*512)
AB_RPC = P // N_CORES          # 1024 pc-query rows per core per batch
BA_RPC = Q_PAD // N_CORES      # 1152 mesh-query rows per core per batch
RT_AB = AB_RPC // 128          # 8 row-tiles
RT_BA = BA_RPC // 128          # 9 row-tiles
CHUNK = 512                    # matmul moving-operand width (ISA max)
N_RT = N_BATCH * (RT_AB + RT_BA)          # 34 row-tiles per core
Q_COLS = N_BATCH * (AB_RPC + BA_RPC)      # 4352
C_COLS = N_BATCH * (Q + P)                # 34434 (exact, no column padding)
GROUP = 2               # 512-wide matmuls per psum group (2 banks)
PSUM_BUFS = 4           # psum groups in flight
WORK_BUFS = 16          # strip/half pool depth
W_BUFS = 32             # 512-wide partial-min tile pool depth


def _chunk_widths(total):
    out = [CHUNK] * (total // CHUNK)
    if total % CHUNK:
        out.append(total % CHUNK)
    return out

# Route weights for psum groups: D = DVE reduce from PSUM, A = ACT egress +
# DVE TT-min, G = ACT egress + GPSIMD TT-min. Chosen to balance DVE/ACT/GPSIMD
# busy time (see module docstring); realized via error-diffusion in _make_routes.
ROUTE_WEIGHTS = {"D": 0.21, "A": 0.79}


def _make_routes(n, weights):
    """Deterministic error-diffusion schedule hitting the weight ratios."""
    acc = dict.fromkeys(weights, 0.0)
    out = []
    for _ in range(n):
        for k in acc:
            acc[k] += weights[k]
        k = max(acc, key=lambda k: acc[k])
        acc[k] -= 1.0
        out.append(k)
    return out

_F32 = mybir.dt.float32
_F16 = mybir.dt.float16
_BF16 = mybir.dt.bfloat16
_BF16_NP = ml_dtypes.bfloat16


def _build_nc(weights=None, group=GROUP, psum_bufs=PSUM_BUFS, work_bufs=WORK_BUFS,
              w_bufs=W_BUFS, repeat=1):
    weights = weights or ROUTE_WEIGHTS
    nc = bacc.Bacc("TRN2", target_bir_lowering=False)
    q_d = nc.dram_tensor("queries", [KDIM, Q_COLS], _BF16, kind="ExternalInput")
    c_d = nc.dram_tensor("cands", [KDIM, C_COLS], _BF16, kind="ExternalInput")
    out_d = nc.dram_tensor("rowmins", [128, N_RT], _F32, kind="ExternalOutput")

    # Per-group route schedule. Ragged (non-1024) groups are forced to route D
    # (reduce handles any width); the error-diffusion schedule covers the rest.
    orient_widths = {"ab": _chunk_widths(Q), "ba": _chunk_widths(P)}
    n_even_groups = 0
    for b in range(N_BATCH):
        for orient in ("ab", "ba"):
            cw = orient_widths[orient]
            n_rt = RT_AB if orient == "ab" else RT_BA
            ngroups = (len(cw) + group - 1) // group
            for rt in range(n_rt):
                for g in range(ngroups):
                    ws = cw[g * group : (g + 1) * group]
                    if len(ws) == group and all(w == CHUNK for w in ws):
                        n_even_groups += 1
    routes = _make_routes(n_even_groups, weights)

    gidx = 0

    def tree_combine(pend, eng):
        """Pairwise TT-min a list of same-width fp16 tiles down to one."""
        while len(pend) > 1:
            nxt = []
            for i in range(0, len(pend) - 1, 2):
                eng.tensor_tensor(
                    pend[i][:], pend[i][:], pend[i + 1][:], op=mybir.AluOpType.min
                )
                nxt.append(pend[i])
            if len(pend) % 2:
                nxt.append(pend[-1])
            pend = nxt
        return pend

    with TileContext(nc) as tc:
        with (
            tc.tile_pool(name="const", bufs=1) as cpool,
            tc.tile_pool(name="psum", bufs=psum_bufs, space="PSUM") as ppool,
            tc.tile_pool(name="work", bufs=work_bufs) as wpool,
            tc.tile_pool(name="wtiles", bufs=w_bufs) as tpool,
        ):
            qt = cpool.tile([KDIM, Q_COLS], _BF16)
            ct = cpool.tile([KDIM, C_COLS], _BF16)
            nc.sync.dma_start(out=qt[:], in_=q_d[:])
            # per-section DMAs so the first row-tiles' matmuls start early
            sec = 0
            for w in (Q, P, Q, P):
                nc.sync.dma_start(
                    out=ct[:, sec : sec + w], in_=c_d[:, sec : sec + w]
                )
                sec += w
            mins = cpool.tile([128, N_RT], _F32)

            for _rep in range(repeat):
                t = 0
                qoff = 0
                coff = 0
                for b in range(N_BATCH):
                    for orient in ("ab", "ba"):
                        n_rt = RT_AB if orient == "ab" else RT_BA
                        cw = orient_widths[orient]
                        coffs = [coff + sum(cw[:i]) for i in range(len(cw))]
                        ngroups = (len(cw) + group - 1) // group
                        for rt in range(n_rt):
                            lhsT = qt[:, qoff + rt * 128 : qoff + (rt + 1) * 128]
                            strip = wpool.tile([128, ngroups + 1], _F32, tag="strip")
                            scol = 0
                            pend_d = []
                            pend1024 = []
                            pending_half = None
                            for g in range(ngroups):
                                ws = cw[g * group : (g + 1) * group]
                                width = sum(ws)
                                ps = ppool.tile([128, group * CHUNK], _F32)
                                off = 0
                                for k, w in enumerate(ws):
                                    rhs = ct[
                                        :, coffs[g * group + k] : coffs[g * group + k] + w
                                    ]
                                    nc.tensor.matmul(
                                        ps[:, off : off + w],
                                        lhsT,
                                        rhs,
                                        start=True,
                                        stop=True,
                                    )
                                    off += w
                                even = len(ws) == group and all(w == CHUNK for w in ws)
                                if even:
                                    route = routes[gidx % len(routes)]
                                    gidx += 1
                                else:
                                    route = "D"
                                if route == "D":
                                    nc.vector.tensor_reduce(
                                        strip[:, scol : scol + 1],
                                        ps[:, :width],
                                        axis=mybir.AxisListType.X,
                                        op=mybir.AluOpType.min,
                                    )
                                    scol += 1
                                    continue
                                half = wpool.tile([128, group * CHUNK], _F16, tag="half")
                                nc.scalar.copy(half[:], ps[:])
                                if pending_half is None:
                                    pending_half = half
                                else:
                                    wt = tpool.tile([128, 2 * CHUNK], _F16, tag="w1024")
                                    nc.vector.tensor_tensor(
                                        wt[:],
                                        pending_half[:],
                                        half[:],
                                        op=mybir.AluOpType.min,
                                    )
                                    pend1024.append(wt)
                                    pending_half = None
                            pend1024 = tree_combine(pend1024, nc.vector)
                            if pending_half is not None:
                                wt = tpool.tile([128, CHUNK], _F16, tag="w512")
                                nc.vector.tensor_tensor(
                                    wt[:],
                                    pending_half[:, :CHUNK],
                                    pending_half[:, CHUNK:],
                                    op=mybir.AluOpType.min,
                                )
                                pend_d.append(wt)
                            if pend1024:
                                wt = tpool.tile([128, CHUNK], _F16, tag="w512")
                                nc.vector.tensor_tensor(
                                    wt[:],
                                    pend1024[0][:, :CHUNK],
                                    pend1024[0][:, CHUNK:],
                                    op=mybir.AluOpType.min,
                                )
                                pend_d.append(wt)
                            rem = tree_combine(pend_d, nc.vector)
                            if len(rem) == 2:
                                nc.vector.tensor_tensor(
                                    rem[0][:], rem[0][:], rem[1][:], op=mybir.AluOpType.min
                                )
                                rem = rem[:1]
                            if rem:
                                nc.vector.tensor_reduce(
                                    strip[:, scol : scol + 1],
                                    rem[0][:],
                                    axis=mybir.AxisListType.X,
                                    op=mybir.AluOpType.min,
                                )
                                scol += 1
                            nc.vector.tensor_reduce(
                                mins[:, t : t + 1],
                                strip[:, :scol],
                                axis=mybir.AxisListType.X,
                                op=mybir.AluOpType.min,
                            )
                            t += 1
                        qoff += n_rt * 128
                        coff += sum(cw)
            nc.sync.dma_start(out=out_d[:], in_=mins[:])
    nc.compile()
    return nc


_NC_CACHE = None


def _get_nc():
    global _NC_CACHE
    if _NC_CACHE is None:
        _NC_CACHE = _build_nc()
    return _NC_CACHE


class _Runner:
    """Persistent jitted shard_map runner (mirrors bass2jax.run_bass_via_pjrt
    but caches the jitted executable so repeated kernel() calls skip retrace)."""

    def __init__(self, nc, n_cores=N_CORES):
        import jax
        from jax.sharding import Mesh, PartitionSpec
        from jax.experimental.shard_map import shard_map
        from concourse import bass2jax

        bass2jax.install_neuronx_cc_hook()
        self._jax = jax
        self.n_cores = n_cores
        part_name = nc.partition_id_tensor.name if nc.partition_id_tensor else None
        in_names, out_names, out_avals, zero_shapes = [], [], [], []
        for alloc in nc.m.functions[0].allocations:
            if not isinstance(alloc, mybir.MemoryLocationSet):
                continue
            name = alloc.memorylocations[0].name
            if alloc.kind == "ExternalInput":
                if name != part_name:
                    in_names.append(name)
            elif alloc.kind == "ExternalOutput":
                out_names.append(name)
                shape = tuple(alloc.tensor_shape)
                dtype = mybir.dt.np(alloc.dtype)
                out_avals.append(jax.core.ShapedArray(shape, dtype))
                zero_shapes.append((shape, dtype))
        self.in_names, self.out_names = in_names, out_names
        self.out_shapes = [s for s, _ in zero_shapes]
        self.zero_shapes = zero_shapes
        n_params = len(in_names)
        all_names = in_names + out_names
        if part_name is not None:
            all_names = all_names + [part_name]

        def _body(*args):
            operands = list(args)
            if part_name is not None:
                operands.append(bass2jax.partition_id_tensor())
            return tuple(
                bass2jax._bass_exec_p.bind(
                    *operands,
                    out_avals=tuple(out_avals),
                    in_names=tuple(all_names),
                    out_names=tuple(out_names),
                    lowering_input_output_aliases=(),
                    sim_require_finite=True,
                    sim_require_nnan=True,
                    nc=nc,
                )
            )

        devices = jax.devices()[:n_cores]
        mesh = Mesh(np.asarray(devices), ("core",))
        n_out = len(out_names)
        self._fn = jax.jit(
            shard_map(
                _body,
                mesh=mesh,
                in_specs=(PartitionSpec("core"),) * (n_params + n_out),
                out_specs=(PartitionSpec("core"),) * n_out,
                check_rep=False,
            ),
            donate_argnums=tuple(range(n_params, n_params + n_out)),
            keep_unused=True,
        )

    def __call__(self, in_maps):
        concat_in = [
            np.concatenate([np.asarray(m[name]) for m in in_maps], axis=0)
            for name in self.in_names
        ]
        zeros = [
            np.zeros((self.n_cores * s[0], *s[1:]), d) for s, d in self.zero_shapes
        ]
        outs = self._fn(*concat_in, *zeros)
        self._jax.block_until_ready(outs)
        results = []
        for c in range(self.n_cores):
            results.append(
                {
                    name: np.asarray(outs[i]).reshape(
                        self.n_cores, *self.out_shapes[i]
                    )[c]
                    for i, name in enumerate(self.out_names)
                }
            )
        return results


_RUNNER_CACHE = None


def _get_runner():
    global _RUNNER_CACHE
    if _RUNNER_CACHE is None:
        _RUNNER_CACHE = _Runner(_get_nc())
    return _RUNNER_CACHE


def _upsample_last(x):
    """[..., W] -> [..., 2W-1] midpoint refinement (align_corners=True)."""
    mid = np.float32(0.5) * (x[..., :-1] + x[..., 1:])
    w = x.shape[-1]
    out = np.zeros(x.shape[:-1] + (2 * w - 1,), x.dtype)
    out[..., 0::2] = x
    out[..., 1::2] = mid
    return out


def _split(x):
    """f32 -> (hi, lo) bf16 pair with hi + lo ~= x."""
    h32 = x.astype(_BF16_NP).astype(np.float32)
    lo = (x - h32).astype(_BF16_NP)
    return h32.astype(_BF16_NP), lo


def _fill_queries(dst, pts, n2):
    """dst: [KDIM, n] bf16; pts: [n, 3] f32 queries; n2: [n] query norms."""
    h, l = _split(pts.T)                 # [3, n] each
    dst[0:3] = h
    dst[3:6] = h
    dst[6:9] = l
    dst[9] = _BF16_NP(1.0)
    dst[10] = _BF16_NP(1.0)
    n2h, n2l = _split(n2)
    dst[11] = n2h
    dst[12] = n2l


def _fill_cands(dst, pts, n2):
    """dst: [KDIM, n] bf16; pts: [n, 3] f32 candidates; n2: [n] cand norms."""
    h, l = _split(-2.0 * pts.T)          # exact *(-2) before split
    dst[0:3] = h
    dst[3:6] = l
    dst[6:9] = h
    n2h, n2l = _split(n2)
    dst[9] = n2h
    dst[10] = n2l
    dst[11] = _BF16_NP(1.0)
    dst[12] = _BF16_NP(1.0)


def _prep_inputs(vertices, pc):
    """Host prep: mesh refinement + augmented query/candidate matrices."""
    v = np.asarray(vertices, dtype=np.float32)
    a = np.asarray(pc, dtype=np.float32)                     # [n, P, 3]
    v = _upsample_last(v)                                    # refine W
    v = _upsample_last(v.swapaxes(-1, -2)).swapaxes(-1, -2)  # refine H
    top = v.reshape(N_BATCH, 3, -1).transpose(0, 2, 1)       # [n, Q, 3]

    a2 = np.sum(a * a, axis=-1)        # [n, P]
    b2 = np.sum(top * top, axis=-1)    # [n, Q]

    queries = [np.empty((KDIM, Q_COLS), dtype=_BF16_NP) for _ in range(N_CORES)]
    cands = np.empty((KDIM, C_COLS), dtype=_BF16_NP)
    qoff = 0
    coff = 0
    for b in range(N_BATCH):
        # --- ab: queries = pc points, candidates = mesh points ---
        for c in range(N_CORES):
            sl = slice(c * AB_RPC, (c + 1) * AB_RPC)
            _fill_queries(queries[c][:, qoff : qoff + AB_RPC], a[b, sl], a2[b, sl])
        _fill_cands(cands[:, coff : coff + Q], top[b], b2[b])
        qoff += AB_RPC
        coff += Q
        # --- ba: queries = mesh points, candidates = pc points ---
        top_pad = np.zeros((Q_PAD, 3), dtype=np.float32)
        top_pad[:Q] = top[b]
        b2_pad = np.zeros(Q_PAD, dtype=np.float32)
        b2_pad[:Q] = b2[b]
        for c in range(N_CORES):
            sl = slice(c * BA_RPC, (c + 1) * BA_RPC)
            _fill_queries(queries[c][:, qoff : qoff + BA_RPC], top_pad[sl], b2_pad[sl])
        _fill_cands(cands[:, coff : coff + P], a[b], a2[b])
        qoff += BA_RPC
        coff += P
    return queries, cands


def _combine(results):
    """Host combine: gather per-row mins (true distances), drop pads, mean."""
    ab_min = np.empty((N_BATCH, P), dtype=np.float32)
    ba_min = np.empty((N_BATCH, Q_PAD), dtype=np.float32)
    for c in range(N_CORES):
        rm = results[c]["rowmins"]                   # [128, N_RT]
        t = 0
        for b in range(N_BATCH):
            for rt in range(RT_AB):
                lo = c * AB_RPC + rt * 128
                ab_min[b, lo : lo + 128] = rm[:, t]
                t += 1
            for rt in range(RT_BA):
                lo = c * BA_RPC + rt * 128
                ba_min[b, lo : lo + 128] = rm[:, t]
                t += 1
    return np.float32(np.mean(ab_min) + np.mean(ba_min[:, :Q]))


def kernel(vertices, pc):
    queries, cands = _prep_inputs(vertices, pc)
    in_maps = [{"queries": queries[c], "cands": cands} for c in range(N_CORES)]
    try:
        results = _get_runner()(in_maps)
    except Exception:
        try:
            results = _get_runner()(in_maps)  # retry once (transient NRT errors)
        except Exception:
            # fallback: reference SPMD path (slower per call, same program)
            res = run_bass_kernel_spmd(
                _get_nc(), in_maps, core_ids=list(range(N_CORES))
            )
            results = res.results
    return np.asarray(_combine(results), dtype=np.float32)

